# revision 10
# baseline (speedup 1.0000x reference)
"""Trainium2 Bass kernel for nn_DiffusionCNN (submanifold sparse 3x3x3 CNN).

Strategy (8-core SPMD, no collectives):
  - Shard the voxel dim N=200000 into 8 contiguous blocks of 25000 (voxels are
    sorted by linear grid index, so neighbor indices are within ~±1700 rows).
  - Each core computes h1 = silu(conv1(x)) over its block plus a halo, writes
    it (bf16) to a private DRAM table, then computes the rest of the net for
    its own 25000 rows.  Halo compute is replicated instead of exchanged.
  - Neighbor gathers run on-device via the SWDGE dma_gather (transpose mode):
    gathered rows land channel-major ([128ch x rows]) and feed the PE matmuls
    directly.  Invalid neighbors gather a dedicated all-zero row.
  - All matmuls in bf16 with fp32 PSUM accumulation (measured end-to-end
    relative error ~4e-3 vs the fp32 reference).

Host-side work is limited to sharding: slicing inputs, remapping neighbor
indices to per-core local tables (int16), packing weights, and re-assembling
the output.  The time embedding concat (N x 6 sin/cos) is computed on the
host as input marshalling; convs / matmuls / gathers all run on device.
"""

import numpy as np
import ml_dtypes

# ---------------------------------------------------------------- constants
N = 200000
PER = 25000
NCORES = 8
C = 128
K = 27
TEMB = 6
IN_CH = 7  # features(1) + sin/cos(6)

TILE = 512
NT1 = 56                 # h1 tiles per core
NT2 = 49                 # output tiles per core
M_H1 = NT1 * TILE        # 28672 h1 rows computed per core (incl. halo + pad)
M_OUT = NT2 * TILE       # 25088 output rows per core (25000 + pad)
XT = 31424               # x table rows (row 0 = zeros)
H1T = 28736              # h1 table rows (row 0 = zeros, then M_H1 rows)
KI = K * TILE            # indices per gather instruction (13824)

_bf16 = ml_dtypes.bfloat16

# CoreSim has no Silu table; when True, emit silu as x*sigmoid(x) instead.
SILU_VIA_SIGMOID = False


# ------------------------------------------------------------- device program
def _build_program():
    import concourse.bass as bass
    import concourse.mybir as mybir
    import concourse.tile as tile
    from concourse import bacc
    from concourse.masks import make_identity

    bf = mybir.dt.bfloat16
    f32 = mybir.dt.float32
    i16 = mybir.dt.int16
    AF = mybir.ActivationFunctionType

    nc = bacc.Bacc("TRN2", target_bir_lowering=False, debug=False)

    x_tab = nc.dram_tensor("x_tab", [XT, C], bf, kind="ExternalInput")
    i1 = nc.dram_tensor("i1", [128, NT1 * KI // 16], i16, kind="ExternalInput")
    i2 = nc.dram_tensor("i2", [128, NT2 * KI // 16], i16, kind="ExternalInput")
    w1 = nc.dram_tensor("w1", [C, K * C], bf, kind="ExternalInput")
    w2 = nc.dram_tensor("w2", [C, K * C], bf, kind="ExternalInput")
    w3 = nc.dram_tensor("w3", [C, C], bf, kind="ExternalInput")
    w4 = nc.dram_tensor("w4", [C, 16], bf, kind="ExternalInput")
    b1 = nc.dram_tensor("b1", [C, 1], f32, kind="ExternalInput")
    b2 = nc.dram_tensor("b2", [C, 1], f32, kind="ExternalInput")
    b3 = nc.dram_tensor("b3", [C, 1], f32, kind="ExternalInput")
    b4 = nc.dram_tensor("b4", [1, 1], f32, kind="ExternalInput")
    outd = nc.dram_tensor("out", [M_OUT], f32, kind="ExternalOutput")
    h1tab = nc.dram_tensor("h1_tab", [H1T, C], bf, kind="Internal")

    with tile.TileContext(nc) as tc:
        with (
            tc.tile_pool(name="const", bufs=1) as constp,
            tc.tile_pool(name="idx", bufs=3) as idxp,
            tc.tile_pool(name="gat", bufs=3) as gatp,
            tc.tile_pool(name="act", bufs=3) as actp,
            tc.tile_pool(name="stage", bufs=3) as stagep,
            tc.tile_pool(name="osb", bufs=1) as outp,
            tc.tile_pool(name="psacc", bufs=3, space="PSUM") as psacc,
            tc.tile_pool(name="pstr", bufs=2, space="PSUM") as pstr,
            tc.tile_pool(name="psout", bufs=2, space="PSUM") as psout,
        ):
            w1_sb = constp.tile([C, K * C], bf, tag="w1")
            nc.sync.dma_start(w1_sb[:], w1[:])
            w2_sb = constp.tile([C, K * C], bf, tag="w2")
            nc.sync.dma_start(w2_sb[:], w2[:])
            w3_sb = constp.tile([C, C], bf, tag="w3")
            nc.sync.dma_start(w3_sb[:], w3[:])
            w4_sb = constp.tile([C, 16], bf, tag="w4")
            nc.sync.dma_start(w4_sb[:], w4[:])
            b1_sb = constp.tile([C, 1], f32, tag="b1")
            nc.sync.dma_start(b1_sb[:], b1[:])
            b2_sb = constp.tile([C, 1], f32, tag="b2")
            nc.sync.dma_start(b2_sb[:], b2[:])
            b3_sb = constp.tile([C, 1], f32, tag="b3")
            nc.sync.dma_start(b3_sb[:], b3[:])
            b4_sb = constp.tile([1, 1], f32, tag="b4")
            nc.sync.dma_start(b4_sb[:], b4[:])
            ident = constp.tile([C, C], bf, tag="ident")
            make_identity(nc, ident[:])
            zrow = constp.tile([1, C], bf, tag="zrow")
            nc.vector.memset(zrow[:], 0.0)
            nc.sync.dma_start(h1tab[0:1, :], zrow[0:1, :])

            out_sb = outp.tile([1, M_OUT], f32, tag="out")

            nchunk = TILE // 128

            def gather(src, idx_dram, t):
                it = idxp.tile([128, KI // 16], i16, tag="it")
                nc.sync.dma_start(
                    it[:], idx_dram[:, t * (KI // 16):(t + 1) * (KI // 16)]
                )
                g = gatp.tile([128, KI], bf, tag="g")
                nc.gpsimd.dma_gather(
                    out_ap=g[:].rearrange("p (o n) -> p o n", o=1),
                    in_ap=src[:, :],
                    idxs_ap=it[:, :],
                    num_idxs=KI,
                    num_idxs_reg=KI,
                    elem_size=C,
                    transpose=True,
                    single_packet=False,
                )
                return g

            def act_silu(dst, ps, bias_ap):
                if not SILU_VIA_SIGMOID:
                    nc.scalar.activation(dst[:], ps[:], AF.Silu, bias=bias_ap)
                else:
                    xb = actp.tile([C, TILE], bf, tag="xb")
                    nc.scalar.activation(xb[:], ps[:], AF.Identity, bias=bias_ap)
                    sg = actp.tile([C, TILE], bf, tag="sg")
                    nc.scalar.activation(sg[:], ps[:], AF.Sigmoid, bias=bias_ap)
                    nc.vector.tensor_tensor(
                        dst[:], xb[:], sg[:], op=mybir.AluOpType.mult
                    )

            def conv_acc(g, w_sb):
                ps = psacc.tile([C, TILE], f32, tag="acc")
                for k in range(K):
                    nc.tensor.matmul(
                        ps[:],
                        lhsT=w_sb[:, C * k:C * (k + 1)],
                        rhs=g[:, TILE * k:TILE * (k + 1)],
                        start=(k == 0),
                        stop=(k == K - 1),
                    )
                return ps

            # ---------------- phase 1: h1 = silu(conv1(x)) -> h1 table ------
            for t in range(NT1):
                g = gather(x_tab, i1, t)
                ps = conv_acc(g, w1_sb)
                h1T = actp.tile([C, TILE], bf, tag="h")
                act_silu(h1T, ps, b1_sb[:, 0:1])
                # transpose [ch, rows] -> [rows, ch] for the row-major table
                pt = pstr.tile([C, TILE], bf, tag="tr")
                for cch in range(nchunk):
                    nc.tensor.matmul(
                        pt[:, 128 * cch:128 * (cch + 1)],
                        lhsT=h1T[:, 128 * cch:128 * (cch + 1)],
                        rhs=ident[:],
                        is_transpose=True,
                        start=(cch == 0),
                        stop=(cch == nchunk - 1),
                    )
                st = stagep.tile([C, TILE], bf, tag="st")
                nc.vector.tensor_copy(st[:], pt[:])
                r0 = 1 + t * TILE
                # physical row of logical in-tile row r is 4*(r%128) + r//128,
                # making this store contiguous per partition (host compensates
                # in the conv2 index values).
                nc.sync.dma_start(
                    h1tab[r0:r0 + TILE, :].rearrange("(p c) e -> p (c e)", c=nchunk),
                    st[:],
                )

            # ---------------- phase 2: conv2 + pointwise MLP ---------------
            for t in range(NT2):
                g = gather(h1tab, i2, t)
                ps = conv_acc(g, w2_sb)
                h2 = actp.tile([C, TILE], bf, tag="h")
                act_silu(h2, ps, b2_sb[:, 0:1])
                ps3 = psacc.tile([C, TILE], f32, tag="acc")
                nc.tensor.matmul(ps3[:], lhsT=w3_sb[:], rhs=h2[:], start=True, stop=True)
                h3 = actp.tile([C, TILE], bf, tag="h")
                act_silu(h3, ps3, b3_sb[:, 0:1])
                ps4 = psout.tile([1, TILE], f32, tag="o")
                nc.tensor.matmul(
                    ps4[:], lhsT=w4_sb[:, 0:1], rhs=h3[:], start=True, stop=True
                )
                nc.scalar.activation(
                    out_sb[0:1, t * TILE:(t + 1) * TILE],
                    ps4[:],
                    AF.Identity,
                    bias=b4_sb[0:1, 0:1],
                )

            nc.sync.dma_start(outd[None, :], out_sb[0:1, :])

    nc.compile()
    return nc


_NC_CACHE = {}


def _get_nc():
    if "nc" not in _NC_CACHE:
        _NC_CACHE["nc"] = _build_program()
    return _NC_CACHE["nc"]


# ------------------------------------------------------------------ host prep
def _sinusoidal(t):
    half = TEMB // 2
    freqs = (np.float32(2.0) ** np.arange(half, dtype=np.float32)) * np.float32(np.pi)
    ang = t.astype(np.float32)[:, None] * freqs[None, :]
    return np.concatenate([np.sin(ang), np.cos(ang)], -1).astype(np.float32)


def _wrap_idx(idx_all):
    """[27, T*TILE] int -> [128, T*KI/16] int16 in the SWDGE index layout
    (per tile: k-major flat, wrapped into 16 partitions, replicated x8)."""
    T = idx_all.shape[1] // TILE
    a = idx_all.reshape(K, T, TILE).transpose(1, 0, 2).reshape(T, KI // 16, 16)
    a = a.transpose(2, 0, 1).reshape(16, T * (KI // 16))
    return np.tile(a, (8, 1)).astype(np.int16)


def _phys_h1_row(j):
    """Logical local h1 row -> physical row in the h1 table (pre-swizzle that
    makes the device-side table store contiguous)."""
    t = j // TILE
    r = j % TILE
    return t * TILE + 4 * (r % 128) + r // 128


def _prep_core(core, x_full, nidx):
    s = core * PER
    e = s + PER

    sub2 = nidx[:, s:e]
    v2 = sub2[sub2 < N]
    lo1 = int(min(v2.min(), s))
    hi1 = int(max(v2.max() + 1, e))
    n1 = hi1 - lo1
    assert n1 <= M_H1, (core, n1)

    sub1 = nidx[:, lo1:hi1]
    v1 = sub1[sub1 < N]
    lo0 = int(min(v1.min(), lo1))
    hi0 = int(max(v1.max() + 1, hi1))
    n0 = hi0 - lo0
    assert n0 + 1 <= XT, (core, n0)

    # x table (row 0 zeros)
    x_tab = np.zeros((XT, C), _bf16)
    x_tab[1:1 + n0, :IN_CH] = x_full[lo0:hi0].astype(_bf16)

    # conv1 indices: for h1 rows [lo1, hi1) (padded to M_H1), 27 offsets
    I1 = np.zeros((K, M_H1), np.int32)
    g = sub1.astype(np.int64)
    valid = g < N
    I1[:, :n1] = np.where(valid, g - lo0 + 1, 0)

    # conv2 indices: for out rows [s, e) (padded to M_OUT), into the h1 table
    I2 = np.zeros((K, M_OUT), np.int32)
    g2 = sub2.astype(np.int64)
    valid2 = g2 < N
    jloc = g2 - lo1  # logical local h1 row, in [0, n1)
    I2[:, :PER] = np.where(valid2, 1 + _phys_h1_row(jloc), 0)

    assert I1.max() < 32768 and I2.max() < 32768
    return {
        "x_tab": x_tab,
        "i1": _wrap_idx(I1),
        "i2": _wrap_idx(I2),
    }


def _prep_shared(W1, b1, W2, b2, W3, b3, W4, b4):
    W1p = np.zeros((K, C, C), np.float32)
    W1p[:, :IN_CH, :] = W1
    w1d = np.ascontiguousarray(
        W1p.transpose(1, 0, 2).reshape(C, K * C)
    ).astype(_bf16)
    w2d = np.ascontiguousarray(
        W2.transpose(1, 0, 2).reshape(C, K * C)
    ).astype(_bf16)
    w3d = np.ascontiguousarray(W3).astype(_bf16)
    w4d = np.zeros((C, 16), _bf16)
    w4d[:, 0] = W4[:, 0].astype(_bf16)
    return {
        "w1": w1d,
        "w2": w2d,
        "w3": w3d,
        "w4": w4d,
        "b1": np.ascontiguousarray(b1.reshape(C, 1), dtype=np.float32),
        "b2": np.ascontiguousarray(b2.reshape(C, 1), dtype=np.float32),
        "b3": np.ascontiguousarray(b3.reshape(C, 1), dtype=np.float32),
        "b4": np.ascontiguousarray(b4.reshape(1, 1), dtype=np.float32),
    }


def _run_pjrt(nc, in_maps, reps=0):
    """Execute the Bass program on the 8 axon-tunneled cores via PJRT.

    Mirrors bass2jax.run_bass_via_pjrt's multi-core branch, but keeps the
    jitted callable + device-resident inputs so repeated executions can be
    timed (reps > 0)."""
    import time as _time
    import jax
    from jax.sharding import Mesh, NamedSharding, PartitionSpec
    from jax.experimental.shard_map import shard_map
    import concourse.mybir as mybir
    from concourse import bass2jax

    bass2jax.install_neuronx_cc_hook()

    n_cores = len(in_maps)
    partition_name = (
        nc.partition_id_tensor.name if nc.partition_id_tensor else None
    )
    in_names, out_names, out_avals, zero_outs = [], [], [], []
    for alloc in nc.m.functions[0].allocations:
        if not isinstance(alloc, mybir.MemoryLocationSet):
            continue
        name = alloc.memorylocations[0].name
        if alloc.kind == "ExternalInput":
            if name != partition_name:
                in_names.append(name)
        elif alloc.kind == "ExternalOutput":
            shape = tuple(alloc.tensor_shape)
            dtype = mybir.dt.np(alloc.dtype)
            out_names.append(name)
            out_avals.append(jax.core.ShapedArray(shape, dtype))
            zero_outs.append(np.zeros(shape, dtype))
    n_params = len(in_names)
    n_outs = len(out_names)
    all_names = in_names + out_names
    if partition_name is not None:
        all_names = all_names + [partition_name]
    donate = tuple(range(n_params, n_params + n_outs))

    def _body(*args):
        operands = list(args)
        if partition_name is not None:
            operands.append(bass2jax.partition_id_tensor())
        outs = bass2jax._bass_exec_p.bind(
            *operands,
            out_avals=tuple(out_avals),
            in_names=tuple(all_names),
            out_names=tuple(out_names),
            lowering_input_output_aliases=(),
            sim_require_finite=True,
            sim_require_nnan=True,
            nc=nc,
        )
        return tuple(outs)

    devices = jax.devices()[:n_cores]
    mesh = Mesh(np.asarray(devices), ("core",))
    spec = PartitionSpec("core")
    sharded = jax.jit(
        shard_map(_body, mesh=mesh, in_specs=(spec,) * (n_params + n_outs),
                  out_specs=(spec,) * n_outs, check_rep=False),
        donate_argnums=donate,
        keep_unused=True,
    )
    concat_in = [
        np.concatenate([np.asarray(m[name]) for m in in_maps], axis=0)
        for name in in_names
    ]
    sh = NamedSharding(mesh, spec)
    inp_dev = [jax.device_put(a, sh) for a in concat_in]

    def _zeros():
        return [np.zeros((n_cores * z.shape[0], *z.shape[1:]), z.dtype)
                for z in zero_outs]

    out_arrs = sharded(*inp_dev, *_zeros())
    jax.block_until_ready(out_arrs)
    results = [
        {name: np.asarray(out_arrs[i]).reshape(n_cores, *out_avals[i].shape)[c]
         for i, name in enumerate(out_names)}
        for c in range(n_cores)
    ]

    times = []
    for _ in range(reps):
        zs = _zeros()
        t0 = _time.perf_counter()
        o = sharded(*inp_dev, *zs)
        jax.block_until_ready(o)
        times.append(_time.perf_counter() - t0)
    return results, times


def _run(inputs, reps=0):
    features = np.asarray(inputs["features"], np.float32)
    t = np.asarray(inputs["t"])
    nidx = np.asarray(inputs["neighbor_idx"]).astype(np.int32)
    x_full = np.concatenate([features, _sinusoidal(t)], -1)

    shared = _prep_shared(
        np.asarray(inputs["W1"], np.float32), np.asarray(inputs["b1"], np.float32),
        np.asarray(inputs["W2"], np.float32), np.asarray(inputs["b2"], np.float32),
        np.asarray(inputs["W3"], np.float32), np.asarray(inputs["b3"], np.float32),
        np.asarray(inputs["W4"], np.float32), np.asarray(inputs["b4"], np.float32),
    )
    in_maps = []
    for core in range(NCORES):
        m = _prep_core(core, x_full, nidx)
        m.update(shared)
        in_maps.append(m)

    nc = _get_nc()
    results, times = _run_pjrt(nc, in_maps, reps=reps)
    out = np.empty((N, 1), np.float32)
    for core in range(NCORES):
        out[core * PER:(core + 1) * PER, 0] = results[core]["out"][:PER]
    return out, times


def kernel(**inputs) -> np.ndarray:
    out, _ = _run(inputs, reps=0)
    return out


# revision 15
# speedup vs baseline: 3.7785x; 3.7785x over previous
"""Trainium2 Bass kernel for nn_DiffusionCNN (submanifold sparse 3x3x3 CNN).

Strategy (8-core SPMD, no collectives):
  - Shard the voxel dim N=200000 into 8 contiguous blocks of 25000 (voxels are
    sorted by linear grid index, so neighbor indices are within ~±1700 rows).
  - Each core computes h1 = silu(conv1(x)) over its block plus a halo, writes
    it (bf16) to a private DRAM table, then computes the rest of the net for
    its own 25000 rows.  Halo compute is replicated instead of exchanged.
  - Neighbor gathers run on-device via the SWDGE dma_gather (transpose mode):
    gathered rows land channel-major ([128ch x rows]) and feed the PE matmuls
    directly.  Invalid neighbors gather a dedicated all-zero row.
  - All matmuls in bf16 with fp32 PSUM accumulation (measured end-to-end
    relative error ~4e-3 vs the fp32 reference).

Host-side work is limited to sharding: slicing inputs, remapping neighbor
indices to per-core local tables (int16), packing weights, and re-assembling
the output.  The time embedding concat (N x 6 sin/cos) is computed on the
host as input marshalling; convs / matmuls / gathers all run on device.
"""

import numpy as np
import ml_dtypes

# ---------------------------------------------------------------- constants
N = 200000
PER = 25000
NCORES = 8
C = 128
K = 27
TEMB = 6
IN_CH = 7  # features(1) + sin/cos(6)

TILE = 512
NT1 = 56                 # h1 tiles per core
NT2 = 49                 # output tiles per core
M_H1 = NT1 * TILE        # 28672 h1 rows computed per core (incl. halo + pad)
M_OUT = NT2 * TILE       # 25088 output rows per core (25000 + pad)
XT = 31424               # x table rows (row 0 = zeros)
H1T = 1 + M_H1           # h1 table rows (row 0 = zeros, then M_H1 rows)
KI = K * TILE            # indices per gather instruction (13824)

_bf16 = ml_dtypes.bfloat16

# CoreSim has no Silu table; when True, emit silu as x*sigmoid(x) instead.
SILU_VIA_SIGMOID = False


# ------------------------------------------------------------- device program
def _build_program(bench_reps=0):
    import concourse.bass as bass
    import concourse.mybir as mybir
    import concourse.tile as tile
    from concourse import bacc
    from concourse.masks import make_identity

    bf = mybir.dt.bfloat16
    f32 = mybir.dt.float32
    i16 = mybir.dt.int16
    AF = mybir.ActivationFunctionType

    nc = bacc.Bacc("TRN2", target_bir_lowering=False, debug=False)

    x_tab = nc.dram_tensor("x_tab", [XT, C], bf, kind="ExternalInput")
    i1 = nc.dram_tensor("i1", [128, NT1 * KI // 16], i16, kind="ExternalInput")
    i2 = nc.dram_tensor("i2", [128, NT2 * KI // 16], i16, kind="ExternalInput")
    w1 = nc.dram_tensor("w1", [C, K * C], bf, kind="ExternalInput")
    w2 = nc.dram_tensor("w2", [C, K * C], bf, kind="ExternalInput")
    w3 = nc.dram_tensor("w3", [C, C], bf, kind="ExternalInput")
    w4 = nc.dram_tensor("w4", [C, 16], bf, kind="ExternalInput")
    b1 = nc.dram_tensor("b1", [C, 1], f32, kind="ExternalInput")
    b2 = nc.dram_tensor("b2", [C, 1], f32, kind="ExternalInput")
    b3 = nc.dram_tensor("b3", [C, 1], f32, kind="ExternalInput")
    b4 = nc.dram_tensor("b4", [1, 1], f32, kind="ExternalInput")
    outd = nc.dram_tensor("out", [M_OUT], f32, kind="ExternalOutput")
    h1tab = nc.dram_tensor("h1_tab", [H1T, C], bf, kind="Internal")

    with tile.TileContext(nc) as tc:
        with (
            tc.tile_pool(name="const", bufs=1) as constp,
            tc.tile_pool(name="idx", bufs=3) as idxp,
            tc.tile_pool(name="gat", bufs=3) as gatp,
            tc.tile_pool(name="act", bufs=3) as actp,
            tc.tile_pool(name="stage", bufs=3) as stagep,
            tc.tile_pool(name="osb", bufs=1) as outp,
            tc.tile_pool(name="psacc", bufs=3, space="PSUM") as psacc,
            tc.tile_pool(name="pstr", bufs=2, space="PSUM") as pstr,
            tc.tile_pool(name="psout", bufs=2, space="PSUM") as psout,
        ):
            w1_sb = constp.tile([C, K * C], bf, tag="w1")
            nc.sync.dma_start(w1_sb[:], w1[:])
            w2_sb = constp.tile([C, K * C], bf, tag="w2")
            nc.sync.dma_start(w2_sb[:], w2[:])
            w3_sb = constp.tile([C, C], bf, tag="w3")
            nc.sync.dma_start(w3_sb[:], w3[:])
            w4_sb = constp.tile([C, 16], bf, tag="w4")
            nc.sync.dma_start(w4_sb[:], w4[:])
            b1_sb = constp.tile([C, 1], f32, tag="b1")
            nc.sync.dma_start(b1_sb[:], b1[:])
            b2_sb = constp.tile([C, 1], f32, tag="b2")
            nc.sync.dma_start(b2_sb[:], b2[:])
            b3_sb = constp.tile([C, 1], f32, tag="b3")
            nc.sync.dma_start(b3_sb[:], b3[:])
            b4_sb = constp.tile([1, 1], f32, tag="b4")
            nc.sync.dma_start(b4_sb[:], b4[:])
            ident = constp.tile([C, C], bf, tag="ident")
            make_identity(nc, ident[:])
            zrow = constp.tile([1, C], bf, tag="zrow")
            nc.vector.memset(zrow[:], 0.0)
            nc.sync.dma_start(h1tab[0:1, :], zrow[0:1, :])

            out_sb = outp.tile([1, M_OUT], f32, tag="out")

            nchunk = TILE // 128

            def gather(src, idx_dram, t):
                it = idxp.tile([128, KI // 16], i16, tag="it")
                nc.sync.dma_start(
                    it[:], idx_dram[:, t * (KI // 16):(t + 1) * (KI // 16)]
                )
                g = gatp.tile([128, KI], bf, tag="g")
                nc.gpsimd.dma_gather(
                    out_ap=g[:].rearrange("p (o n) -> p o n", o=1),
                    in_ap=src[:, :],
                    idxs_ap=it[:, :],
                    num_idxs=KI,
                    num_idxs_reg=KI,
                    elem_size=C,
                    transpose=True,
                    single_packet=False,
                )
                return g

            def act_silu(dst, ps, bias_ap):
                if not SILU_VIA_SIGMOID:
                    nc.scalar.activation(dst[:], ps[:], AF.Silu, bias=bias_ap)
                else:
                    xb = actp.tile([C, TILE], bf, tag="xb")
                    nc.scalar.activation(xb[:], ps[:], AF.Identity, bias=bias_ap)
                    sg = actp.tile([C, TILE], bf, tag="sg")
                    nc.scalar.activation(sg[:], ps[:], AF.Sigmoid, bias=bias_ap)
                    nc.vector.tensor_tensor(
                        dst[:], xb[:], sg[:], op=mybir.AluOpType.mult
                    )

            def conv_acc(g, w_sb):
                ps = psacc.tile([C, TILE], f32, tag="acc")
                for k in range(K):
                    nc.tensor.matmul(
                        ps[:],
                        lhsT=w_sb[:, C * k:C * (k + 1)],
                        rhs=g[:, TILE * k:TILE * (k + 1)],
                        start=(k == 0),
                        stop=(k == K - 1),
                    )
                return ps

            def emit_body():
                # ------------ phase 1: h1 = silu(conv1(x)) -> h1 table ------
                for t in range(NT1):
                    g = gather(x_tab, i1, t)
                    ps = conv_acc(g, w1_sb)
                    h1T = actp.tile([C, TILE], bf, tag="h")
                    act_silu(h1T, ps, b1_sb[:, 0:1])
                    # transpose [ch, rows] -> [rows, ch] for the table
                    pt = pstr.tile([C, TILE], bf, tag="tr")
                    for cch in range(nchunk):
                        nc.tensor.matmul(
                            pt[:, 128 * cch:128 * (cch + 1)],
                            lhsT=h1T[:, 128 * cch:128 * (cch + 1)],
                            rhs=ident[:],
                            is_transpose=True,
                            start=(cch == 0),
                            stop=(cch == nchunk - 1),
                        )
                    st = stagep.tile([C, TILE], bf, tag="st")
                    nc.vector.tensor_copy(st[:], pt[:])
                    r0 = 1 + t * TILE
                    # physical row of logical in-tile row r is
                    # 4*(r%128) + r//128, making this store contiguous per
                    # partition (host compensates in the conv2 indices).
                    nc.sync.dma_start(
                        h1tab[r0:r0 + TILE, :].rearrange(
                            "(p c) e -> p (c e)", c=nchunk),
                        st[:],
                    )

                # ------------ phase 2: conv2 + pointwise MLP ---------------
                for t in range(NT2):
                    g = gather(h1tab, i2, t)
                    ps = conv_acc(g, w2_sb)
                    h2 = actp.tile([C, TILE], bf, tag="h")
                    act_silu(h2, ps, b2_sb[:, 0:1])
                    ps3 = psacc.tile([C, TILE], f32, tag="acc")
                    nc.tensor.matmul(ps3[:], lhsT=w3_sb[:], rhs=h2[:],
                                     start=True, stop=True)
                    h3 = actp.tile([C, TILE], bf, tag="h")
                    act_silu(h3, ps3, b3_sb[:, 0:1])
                    ps4 = psout.tile([1, TILE], f32, tag="o")
                    nc.tensor.matmul(ps4[:], lhsT=w4_sb[:, 0:1], rhs=h3[:],
                                     start=True, stop=True)
                    nc.scalar.activation(
                        out_sb[0:1, t * TILE:(t + 1) * TILE],
                        ps4[:],
                        AF.Identity,
                        bias=b4_sb[0:1, 0:1],
                    )

                nc.sync.dma_start(outd[None, :], out_sb[0:1, :])

            if bench_reps > 0:
                with tc.For_i(0, bench_reps, 1):
                    emit_body()
            else:
                emit_body()

    nc.compile()
    return nc


_NC_CACHE = {}


def _get_nc():
    if "nc" not in _NC_CACHE:
        _NC_CACHE["nc"] = _build_program()
    return _NC_CACHE["nc"]


# ------------------------------------------------------------------ host prep
def _sinusoidal(t):
    half = TEMB // 2
    freqs = (np.float32(2.0) ** np.arange(half, dtype=np.float32)) * np.float32(np.pi)
    ang = t.astype(np.float32)[:, None] * freqs[None, :]
    return np.concatenate([np.sin(ang), np.cos(ang)], -1).astype(np.float32)


def _wrap_idx(idx_all):
    """[27, T*TILE] int -> [128, T*KI/16] int16 in the SWDGE index layout
    (per tile: k-major flat, wrapped into 16 partitions, replicated x8)."""
    T = idx_all.shape[1] // TILE
    a = idx_all.reshape(K, T, TILE).transpose(1, 0, 2).reshape(T, KI // 16, 16)
    a = a.transpose(2, 0, 1).reshape(16, T * (KI // 16))
    return np.tile(a, (8, 1)).astype(np.int16)


def _phys_h1_row(j):
    """Logical local h1 row -> physical row in the h1 table (pre-swizzle that
    makes the device-side table store contiguous)."""
    t = j // TILE
    r = j % TILE
    return t * TILE + 4 * (r % 128) + r // 128


def _prep_core(core, x_full, nidx):
    s = core * PER
    e = s + PER

    sub2 = nidx[:, s:e]
    v2 = sub2[sub2 < N]
    lo1 = int(min(v2.min(), s))
    hi1 = int(max(v2.max() + 1, e))
    n1 = hi1 - lo1
    assert n1 <= M_H1, (core, n1)

    sub1 = nidx[:, lo1:hi1]
    v1 = sub1[sub1 < N]
    lo0 = int(min(v1.min(), lo1))
    hi0 = int(max(v1.max() + 1, hi1))
    n0 = hi0 - lo0
    assert n0 + 1 <= XT, (core, n0)

    # x table (row 0 zeros)
    x_tab = np.zeros((XT, C), _bf16)
    x_tab[1:1 + n0, :IN_CH] = x_full[lo0:hi0].astype(_bf16)

    # conv1 indices: for h1 rows [lo1, hi1) (padded to M_H1), 27 offsets
    I1 = np.zeros((K, M_H1), np.int32)
    g = sub1.astype(np.int64)
    valid = g < N
    I1[:, :n1] = np.where(valid, g - lo0 + 1, 0)

    # conv2 indices: for out rows [s, e) (padded to M_OUT), into the h1 table
    I2 = np.zeros((K, M_OUT), np.int32)
    g2 = sub2.astype(np.int64)
    valid2 = g2 < N
    jloc = g2 - lo1  # logical local h1 row, in [0, n1)
    I2[:, :PER] = np.where(valid2, 1 + _phys_h1_row(jloc), 0)

    assert I1.max() < 32768 and I2.max() < 32768
    return {
        "x_tab": x_tab,
        "i1": _wrap_idx(I1),
        "i2": _wrap_idx(I2),
    }


def _prep_shared(W1, b1, W2, b2, W3, b3, W4, b4):
    W1p = np.zeros((K, C, C), np.float32)
    W1p[:, :IN_CH, :] = W1
    w1d = np.ascontiguousarray(
        W1p.transpose(1, 0, 2).reshape(C, K * C)
    ).astype(_bf16)
    w2d = np.ascontiguousarray(
        W2.transpose(1, 0, 2).reshape(C, K * C)
    ).astype(_bf16)
    w3d = np.ascontiguousarray(W3).astype(_bf16)
    w4d = np.zeros((C, 16), _bf16)
    w4d[:, 0] = W4[:, 0].astype(_bf16)
    return {
        "w1": w1d,
        "w2": w2d,
        "w3": w3d,
        "w4": w4d,
        "b1": np.ascontiguousarray(b1.reshape(C, 1), dtype=np.float32),
        "b2": np.ascontiguousarray(b2.reshape(C, 1), dtype=np.float32),
        "b3": np.ascontiguousarray(b3.reshape(C, 1), dtype=np.float32),
        "b4": np.ascontiguousarray(b4.reshape(1, 1), dtype=np.float32),
    }


def _run_pjrt(nc, in_maps, reps=0):
    """Execute the Bass program on the 8 axon-tunneled cores via PJRT.

    Mirrors bass2jax.run_bass_via_pjrt's multi-core branch, but keeps the
    jitted callable + device-resident inputs so repeated executions can be
    timed (reps > 0)."""
    import time as _time
    import jax
    from jax.sharding import Mesh, NamedSharding, PartitionSpec
    from jax.experimental.shard_map import shard_map
    import concourse.mybir as mybir
    from concourse import bass2jax

    bass2jax.install_neuronx_cc_hook()

    n_cores = len(in_maps)
    partition_name = (
        nc.partition_id_tensor.name if nc.partition_id_tensor else None
    )
    in_names, out_names, out_avals, zero_outs = [], [], [], []
    for alloc in nc.m.functions[0].allocations:
        if not isinstance(alloc, mybir.MemoryLocationSet):
            continue
        name = alloc.memorylocations[0].name
        if alloc.kind == "ExternalInput":
            if name != partition_name:
                in_names.append(name)
        elif alloc.kind == "ExternalOutput":
            shape = tuple(alloc.tensor_shape)
            dtype = mybir.dt.np(alloc.dtype)
            out_names.append(name)
            out_avals.append(jax.core.ShapedArray(shape, dtype))
            zero_outs.append(np.zeros(shape, dtype))
    n_params = len(in_names)
    n_outs = len(out_names)
    all_names = in_names + out_names
    if partition_name is not None:
        all_names = all_names + [partition_name]
    donate = tuple(range(n_params, n_params + n_outs))

    def _body(*args):
        operands = list(args)
        if partition_name is not None:
            operands.append(bass2jax.partition_id_tensor())
        outs = bass2jax._bass_exec_p.bind(
            *operands,
            out_avals=tuple(out_avals),
            in_names=tuple(all_names),
            out_names=tuple(out_names),
            lowering_input_output_aliases=(),
            sim_require_finite=True,
            sim_require_nnan=True,
            nc=nc,
        )
        return tuple(outs)

    devices = jax.devices()[:n_cores]
    mesh = Mesh(np.asarray(devices), ("core",))
    spec = PartitionSpec("core")
    sharded = jax.jit(
        shard_map(_body, mesh=mesh, in_specs=(spec,) * (n_params + n_outs),
                  out_specs=(spec,) * n_outs, check_rep=False),
        donate_argnums=donate,
        keep_unused=True,
    )
    concat_in = [
        np.concatenate([np.asarray(m[name]) for m in in_maps], axis=0)
        for name in in_names
    ]
    sh = NamedSharding(mesh, spec)
    inp_dev = [jax.device_put(a, sh) for a in concat_in]

    def _zeros():
        return [np.zeros((n_cores * z.shape[0], *z.shape[1:]), z.dtype)
                for z in zero_outs]

    out_arrs = sharded(*inp_dev, *_zeros())
    jax.block_until_ready(out_arrs)
    results = [
        {name: np.asarray(out_arrs[i]).reshape(n_cores, *out_avals[i].shape)[c]
         for i, name in enumerate(out_names)}
        for c in range(n_cores)
    ]

    times = []
    for _ in range(reps):
        zs = _zeros()
        t0 = _time.perf_counter()
        o = sharded(*inp_dev, *zs)
        jax.block_until_ready(o)
        times.append(_time.perf_counter() - t0)
    return results, times


def _run(inputs, reps=0):
    features = np.asarray(inputs["features"], np.float32)
    t = np.asarray(inputs["t"])
    nidx = np.asarray(inputs["neighbor_idx"]).astype(np.int32)
    x_full = np.concatenate([features, _sinusoidal(t)], -1)

    shared = _prep_shared(
        np.asarray(inputs["W1"], np.float32), np.asarray(inputs["b1"], np.float32),
        np.asarray(inputs["W2"], np.float32), np.asarray(inputs["b2"], np.float32),
        np.asarray(inputs["W3"], np.float32), np.asarray(inputs["b3"], np.float32),
        np.asarray(inputs["W4"], np.float32), np.asarray(inputs["b4"], np.float32),
    )
    in_maps = []
    for core in range(NCORES):
        m = _prep_core(core, x_full, nidx)
        m.update(shared)
        in_maps.append(m)

    nc = _get_nc()
    results, times = _run_pjrt(nc, in_maps, reps=reps)
    out = np.empty((N, 1), np.float32)
    for core in range(NCORES):
        out[core * PER:(core + 1) * PER, 0] = results[core]["out"][:PER]
    return out, times


def kernel(**inputs) -> np.ndarray:
    out, _ = _run(inputs, reps=0)
    return out


def _prep_in_maps(inputs):
    features = np.asarray(inputs["features"], np.float32)
    t = np.asarray(inputs["t"])
    nidx = np.asarray(inputs["neighbor_idx"]).astype(np.int32)
    x_full = np.concatenate([features, _sinusoidal(t)], -1)
    shared = _prep_shared(
        np.asarray(inputs["W1"], np.float32), np.asarray(inputs["b1"], np.float32),
        np.asarray(inputs["W2"], np.float32), np.asarray(inputs["b2"], np.float32),
        np.asarray(inputs["W3"], np.float32), np.asarray(inputs["b3"], np.float32),
        np.asarray(inputs["W4"], np.float32), np.asarray(inputs["b4"], np.float32),
    )
    in_maps = []
    for core in range(NCORES):
        m = _prep_core(core, x_full, nidx)
        m.update(shared)
        in_maps.append(m)
    return in_maps


def bench(inputs, loop_reps=(1, 26), wall_reps=8):
    """Estimate on-device kernel time by diffing wall times of programs that
    loop the whole body R1 vs R2 times on-device (cancels the ~105ms axon
    RPC floor)."""
    in_maps = _prep_in_maps(inputs)
    walls = {}
    outs = {}
    for R in loop_reps:
        nc = _build_program(bench_reps=R)
        results, times = _run_pjrt(nc, in_maps, reps=wall_reps)
        walls[R] = min(times)
        out = np.empty((N, 1), np.float32)
        for core in range(NCORES):
            out[core * PER:(core + 1) * PER, 0] = results[core]["out"][:PER]
        outs[R] = out
    R1, R2 = loop_reps
    per_iter = (walls[R2] - walls[R1]) / (R2 - R1)
    return per_iter, walls, outs


# revision 16
# speedup vs baseline: 8.3889x; 2.2201x over previous
"""Trainium2 Bass kernel for nn_DiffusionCNN (submanifold sparse 3x3x3 CNN).

Strategy (8-core SPMD, no collectives):
  - Shard the voxel dim N=200000 into 8 contiguous blocks of 25000 (voxels are
    sorted by linear grid index, so neighbor indices are within ~±1700 rows).
  - Each core computes h1 = silu(conv1(x)) over its block plus a halo, writes
    it (bf16) to a private DRAM table, then computes the rest of the net for
    its own 25000 rows.  Halo compute is replicated instead of exchanged.
  - Neighbor gathers run on-device via the SWDGE dma_gather (transpose mode):
    gathered rows land channel-major ([128ch x rows]) and feed the PE matmuls
    directly.  Invalid neighbors gather a dedicated all-zero row.
  - All matmuls in bf16 with fp32 PSUM accumulation (measured end-to-end
    relative error ~4e-3 vs the fp32 reference).

Host-side work is limited to sharding: slicing inputs, remapping neighbor
indices to per-core local tables (int16), packing weights, and re-assembling
the output.  The time embedding concat (N x 6 sin/cos) is computed on the
host as input marshalling; convs / matmuls / gathers all run on device.
"""

import numpy as np
import ml_dtypes

# ---------------------------------------------------------------- constants
N = 200000
PER = 25000
NCORES = 8
C = 128
K = 27
TEMB = 6
IN_CH = 7  # features(1) + sin/cos(6)

TILE = 512
NT1 = 56                 # h1 tiles per core
NT2 = 49                 # output tiles per core
M_H1 = NT1 * TILE        # 28672 h1 rows computed per core (incl. halo + pad)
M_OUT = NT2 * TILE       # 25088 output rows per core (25000 + pad)
# Invalid neighbors gather a zero row.  ~88% of indices are invalid, and the
# SDMA engines serialize same-address reads, so spread the zero reads over a
# block of NZ distinct zero rows at the front of each table.
NZ = 1024
XT = 32384               # x table rows (NZ zero rows + up to ~31350 real)
H1T = NZ + M_H1          # h1 table rows (NZ zero rows + M_H1 rows)
KI = K * TILE            # indices per gather instruction (13824)

_bf16 = ml_dtypes.bfloat16

# CoreSim has no Silu table; when True, emit silu as x*sigmoid(x) instead.
SILU_VIA_SIGMOID = False


# ------------------------------------------------------------- device program
def _build_program(bench_reps=0):
    import concourse.bass as bass
    import concourse.mybir as mybir
    import concourse.tile as tile
    from concourse import bacc
    from concourse.masks import make_identity

    bf = mybir.dt.bfloat16
    f32 = mybir.dt.float32
    i16 = mybir.dt.int16
    AF = mybir.ActivationFunctionType

    nc = bacc.Bacc("TRN2", target_bir_lowering=False, debug=False)

    x_tab = nc.dram_tensor("x_tab", [XT, C], bf, kind="ExternalInput")
    i1 = nc.dram_tensor("i1", [128, NT1 * KI // 16], i16, kind="ExternalInput")
    i2 = nc.dram_tensor("i2", [128, NT2 * KI // 16], i16, kind="ExternalInput")
    w1 = nc.dram_tensor("w1", [C, K * C], bf, kind="ExternalInput")
    w2 = nc.dram_tensor("w2", [C, K * C], bf, kind="ExternalInput")
    w3 = nc.dram_tensor("w3", [C, C], bf, kind="ExternalInput")
    w4 = nc.dram_tensor("w4", [C, 16], bf, kind="ExternalInput")
    b1 = nc.dram_tensor("b1", [C, 1], f32, kind="ExternalInput")
    b2 = nc.dram_tensor("b2", [C, 1], f32, kind="ExternalInput")
    b3 = nc.dram_tensor("b3", [C, 1], f32, kind="ExternalInput")
    b4 = nc.dram_tensor("b4", [1, 1], f32, kind="ExternalInput")
    outd = nc.dram_tensor("out", [M_OUT], f32, kind="ExternalOutput")
    h1tab = nc.dram_tensor("h1_tab", [H1T, C], bf, kind="Internal")

    with tile.TileContext(nc) as tc:
        with (
            tc.tile_pool(name="const", bufs=1) as constp,
            tc.tile_pool(name="idx", bufs=3) as idxp,
            tc.tile_pool(name="gat", bufs=3) as gatp,
            tc.tile_pool(name="act", bufs=3) as actp,
            tc.tile_pool(name="stage", bufs=3) as stagep,
            tc.tile_pool(name="osb", bufs=1) as outp,
            tc.tile_pool(name="psacc", bufs=3, space="PSUM") as psacc,
            tc.tile_pool(name="pstr", bufs=2, space="PSUM") as pstr,
            tc.tile_pool(name="psout", bufs=2, space="PSUM") as psout,
        ):
            w1_sb = constp.tile([C, K * C], bf, tag="w1")
            nc.sync.dma_start(w1_sb[:], w1[:])
            w2_sb = constp.tile([C, K * C], bf, tag="w2")
            nc.sync.dma_start(w2_sb[:], w2[:])
            w3_sb = constp.tile([C, C], bf, tag="w3")
            nc.sync.dma_start(w3_sb[:], w3[:])
            w4_sb = constp.tile([C, 16], bf, tag="w4")
            nc.sync.dma_start(w4_sb[:], w4[:])
            b1_sb = constp.tile([C, 1], f32, tag="b1")
            nc.sync.dma_start(b1_sb[:], b1[:])
            b2_sb = constp.tile([C, 1], f32, tag="b2")
            nc.sync.dma_start(b2_sb[:], b2[:])
            b3_sb = constp.tile([C, 1], f32, tag="b3")
            nc.sync.dma_start(b3_sb[:], b3[:])
            b4_sb = constp.tile([1, 1], f32, tag="b4")
            nc.sync.dma_start(b4_sb[:], b4[:])
            ident = constp.tile([C, C], bf, tag="ident")
            make_identity(nc, ident[:])
            zblk = constp.tile([C, NZ // 128 * C], bf, tag="zblk")
            nc.vector.memset(zblk[:], 0.0)
            nc.sync.dma_start(
                h1tab[0:NZ, :].rearrange("(p c) e -> p (c e)", c=NZ // 128),
                zblk[:])

            out_sb = outp.tile([1, M_OUT], f32, tag="out")

            nchunk = TILE // 128

            def gather(src, idx_dram, t):
                it = idxp.tile([128, KI // 16], i16, tag="it")
                nc.sync.dma_start(
                    it[:], idx_dram[:, t * (KI // 16):(t + 1) * (KI // 16)]
                )
                g = gatp.tile([128, KI], bf, tag="g")
                nc.gpsimd.dma_gather(
                    out_ap=g[:].rearrange("p (o n) -> p o n", o=1),
                    in_ap=src[:, :],
                    idxs_ap=it[:, :],
                    num_idxs=KI,
                    num_idxs_reg=KI,
                    elem_size=C,
                    transpose=True,
                    single_packet=False,
                )
                return g

            def act_silu(dst, ps, bias_ap):
                if not SILU_VIA_SIGMOID:
                    nc.scalar.activation(dst[:], ps[:], AF.Silu, bias=bias_ap)
                else:
                    xb = actp.tile([C, TILE], bf, tag="xb")
                    nc.scalar.activation(xb[:], ps[:], AF.Identity, bias=bias_ap)
                    sg = actp.tile([C, TILE], bf, tag="sg")
                    nc.scalar.activation(sg[:], ps[:], AF.Sigmoid, bias=bias_ap)
                    nc.vector.tensor_tensor(
                        dst[:], xb[:], sg[:], op=mybir.AluOpType.mult
                    )

            def conv_acc(g, w_sb):
                ps = psacc.tile([C, TILE], f32, tag="acc")
                for k in range(K):
                    nc.tensor.matmul(
                        ps[:],
                        lhsT=w_sb[:, C * k:C * (k + 1)],
                        rhs=g[:, TILE * k:TILE * (k + 1)],
                        start=(k == 0),
                        stop=(k == K - 1),
                    )
                return ps

            def emit_body():
                # ------------ phase 1: h1 = silu(conv1(x)) -> h1 table ------
                for t in range(NT1):
                    g = gather(x_tab, i1, t)
                    ps = conv_acc(g, w1_sb)
                    h1T = actp.tile([C, TILE], bf, tag="h")
                    act_silu(h1T, ps, b1_sb[:, 0:1])
                    # transpose [ch, rows] -> [rows, ch] for the table
                    pt = pstr.tile([C, TILE], bf, tag="tr")
                    for cch in range(nchunk):
                        nc.tensor.matmul(
                            pt[:, 128 * cch:128 * (cch + 1)],
                            lhsT=h1T[:, 128 * cch:128 * (cch + 1)],
                            rhs=ident[:],
                            is_transpose=True,
                            start=(cch == 0),
                            stop=(cch == nchunk - 1),
                        )
                    st = stagep.tile([C, TILE], bf, tag="st")
                    nc.vector.tensor_copy(st[:], pt[:])
                    r0 = NZ + t * TILE
                    # physical row of logical in-tile row r is
                    # 4*(r%128) + r//128, making this store contiguous per
                    # partition (host compensates in the conv2 indices).
                    nc.sync.dma_start(
                        h1tab[r0:r0 + TILE, :].rearrange(
                            "(p c) e -> p (c e)", c=nchunk),
                        st[:],
                    )

                # ------------ phase 2: conv2 + pointwise MLP ---------------
                for t in range(NT2):
                    g = gather(h1tab, i2, t)
                    ps = conv_acc(g, w2_sb)
                    h2 = actp.tile([C, TILE], bf, tag="h")
                    act_silu(h2, ps, b2_sb[:, 0:1])
                    ps3 = psacc.tile([C, TILE], f32, tag="acc")
                    nc.tensor.matmul(ps3[:], lhsT=w3_sb[:], rhs=h2[:],
                                     start=True, stop=True)
                    h3 = actp.tile([C, TILE], bf, tag="h")
                    act_silu(h3, ps3, b3_sb[:, 0:1])
                    ps4 = psout.tile([1, TILE], f32, tag="o")
                    nc.tensor.matmul(ps4[:], lhsT=w4_sb[:, 0:1], rhs=h3[:],
                                     start=True, stop=True)
                    nc.scalar.activation(
                        out_sb[0:1, t * TILE:(t + 1) * TILE],
                        ps4[:],
                        AF.Identity,
                        bias=b4_sb[0:1, 0:1],
                    )

                nc.sync.dma_start(outd[None, :], out_sb[0:1, :])

            if bench_reps > 0:
                with tc.For_i(0, bench_reps, 1):
                    emit_body()
            else:
                emit_body()

    nc.compile()
    return nc


_NC_CACHE = {}


def _get_nc():
    if "nc" not in _NC_CACHE:
        _NC_CACHE["nc"] = _build_program()
    return _NC_CACHE["nc"]


# ------------------------------------------------------------------ host prep
def _sinusoidal(t):
    half = TEMB // 2
    freqs = (np.float32(2.0) ** np.arange(half, dtype=np.float32)) * np.float32(np.pi)
    ang = t.astype(np.float32)[:, None] * freqs[None, :]
    return np.concatenate([np.sin(ang), np.cos(ang)], -1).astype(np.float32)


def _wrap_idx(idx_all):
    """[27, T*TILE] int -> [128, T*KI/16] int16 in the SWDGE index layout
    (per tile: k-major flat, wrapped into 16 partitions, replicated x8)."""
    T = idx_all.shape[1] // TILE
    a = idx_all.reshape(K, T, TILE).transpose(1, 0, 2).reshape(T, KI // 16, 16)
    a = a.transpose(2, 0, 1).reshape(16, T * (KI // 16))
    return np.tile(a, (8, 1)).astype(np.int16)


def _phys_h1_row(j):
    """Logical local h1 row -> physical row in the h1 table (pre-swizzle that
    makes the device-side table store contiguous)."""
    t = j // TILE
    r = j % TILE
    return t * TILE + 4 * (r % 128) + r // 128


def _prep_core(core, x_full, nidx):
    s = core * PER
    e = s + PER

    sub2 = nidx[:, s:e]
    v2 = sub2[sub2 < N]
    lo1 = int(min(v2.min(), s))
    hi1 = int(max(v2.max() + 1, e))
    n1 = hi1 - lo1
    assert n1 <= M_H1, (core, n1)

    sub1 = nidx[:, lo1:hi1]
    v1 = sub1[sub1 < N]
    lo0 = int(min(v1.min(), lo1))
    hi0 = int(max(v1.max() + 1, hi1))
    n0 = hi0 - lo0
    assert n0 + 1 <= XT, (core, n0)

    assert n0 <= XT - NZ, (core, n0)
    # x table (rows [0, NZ) zeros)
    x_tab = np.zeros((XT, C), _bf16)
    x_tab[NZ:NZ + n0, :IN_CH] = x_full[lo0:hi0].astype(_bf16)

    rng = np.random.default_rng(12345 + core)
    # conv1 indices: for h1 rows [lo1, hi1) (padded to M_H1), 27 offsets
    I1 = rng.integers(0, NZ, size=(K, M_H1)).astype(np.int32)
    g = sub1.astype(np.int64)
    valid = g < N
    I1[:, :n1] = np.where(valid, g - lo0 + NZ, I1[:, :n1])

    # conv2 indices: for out rows [s, e) (padded to M_OUT), into the h1 table
    I2 = rng.integers(0, NZ, size=(K, M_OUT)).astype(np.int32)
    g2 = sub2.astype(np.int64)
    valid2 = g2 < N
    jloc = g2 - lo1  # logical local h1 row, in [0, n1)
    I2[:, :PER] = np.where(valid2, NZ + _phys_h1_row(jloc), I2[:, :PER])

    assert I1.max() < 32768 and I2.max() < 32768
    return {
        "x_tab": x_tab,
        "i1": _wrap_idx(I1),
        "i2": _wrap_idx(I2),
    }


def _prep_shared(W1, b1, W2, b2, W3, b3, W4, b4):
    W1p = np.zeros((K, C, C), np.float32)
    W1p[:, :IN_CH, :] = W1
    w1d = np.ascontiguousarray(
        W1p.transpose(1, 0, 2).reshape(C, K * C)
    ).astype(_bf16)
    w2d = np.ascontiguousarray(
        W2.transpose(1, 0, 2).reshape(C, K * C)
    ).astype(_bf16)
    w3d = np.ascontiguousarray(W3).astype(_bf16)
    w4d = np.zeros((C, 16), _bf16)
    w4d[:, 0] = W4[:, 0].astype(_bf16)
    return {
        "w1": w1d,
        "w2": w2d,
        "w3": w3d,
        "w4": w4d,
        "b1": np.ascontiguousarray(b1.reshape(C, 1), dtype=np.float32),
        "b2": np.ascontiguousarray(b2.reshape(C, 1), dtype=np.float32),
        "b3": np.ascontiguousarray(b3.reshape(C, 1), dtype=np.float32),
        "b4": np.ascontiguousarray(b4.reshape(1, 1), dtype=np.float32),
    }


def _run_pjrt(nc, in_maps, reps=0):
    """Execute the Bass program on the 8 axon-tunneled cores via PJRT.

    Mirrors bass2jax.run_bass_via_pjrt's multi-core branch, but keeps the
    jitted callable + device-resident inputs so repeated executions can be
    timed (reps > 0)."""
    import time as _time
    import jax
    from jax.sharding import Mesh, NamedSharding, PartitionSpec
    from jax.experimental.shard_map import shard_map
    import concourse.mybir as mybir
    from concourse import bass2jax

    bass2jax.install_neuronx_cc_hook()

    n_cores = len(in_maps)
    partition_name = (
        nc.partition_id_tensor.name if nc.partition_id_tensor else None
    )
    in_names, out_names, out_avals, zero_outs = [], [], [], []
    for alloc in nc.m.functions[0].allocations:
        if not isinstance(alloc, mybir.MemoryLocationSet):
            continue
        name = alloc.memorylocations[0].name
        if alloc.kind == "ExternalInput":
            if name != partition_name:
                in_names.append(name)
        elif alloc.kind == "ExternalOutput":
            shape = tuple(alloc.tensor_shape)
            dtype = mybir.dt.np(alloc.dtype)
            out_names.append(name)
            out_avals.append(jax.core.ShapedArray(shape, dtype))
            zero_outs.append(np.zeros(shape, dtype))
    n_params = len(in_names)
    n_outs = len(out_names)
    all_names = in_names + out_names
    if partition_name is not None:
        all_names = all_names + [partition_name]
    donate = tuple(range(n_params, n_params + n_outs))

    def _body(*args):
        operands = list(args)
        if partition_name is not None:
            operands.append(bass2jax.partition_id_tensor())
        outs = bass2jax._bass_exec_p.bind(
            *operands,
            out_avals=tuple(out_avals),
            in_names=tuple(all_names),
            out_names=tuple(out_names),
            lowering_input_output_aliases=(),
            sim_require_finite=True,
            sim_require_nnan=True,
            nc=nc,
        )
        return tuple(outs)

    devices = jax.devices()[:n_cores]
    mesh = Mesh(np.asarray(devices), ("core",))
    spec = PartitionSpec("core")
    sharded = jax.jit(
        shard_map(_body, mesh=mesh, in_specs=(spec,) * (n_params + n_outs),
                  out_specs=(spec,) * n_outs, check_rep=False),
        donate_argnums=donate,
        keep_unused=True,
    )
    concat_in = [
        np.concatenate([np.asarray(m[name]) for m in in_maps], axis=0)
        for name in in_names
    ]
    sh = NamedSharding(mesh, spec)
    inp_dev = [jax.device_put(a, sh) for a in concat_in]

    def _zeros():
        return [np.zeros((n_cores * z.shape[0], *z.shape[1:]), z.dtype)
                for z in zero_outs]

    out_arrs = sharded(*inp_dev, *_zeros())
    jax.block_until_ready(out_arrs)
    results = [
        {name: np.asarray(out_arrs[i]).reshape(n_cores, *out_avals[i].shape)[c]
         for i, name in enumerate(out_names)}
        for c in range(n_cores)
    ]

    times = []
    for _ in range(reps):
        zs = _zeros()
        t0 = _time.perf_counter()
        o = sharded(*inp_dev, *zs)
        jax.block_until_ready(o)
        times.append(_time.perf_counter() - t0)
    return results, times


def _run(inputs, reps=0):
    features = np.asarray(inputs["features"], np.float32)
    t = np.asarray(inputs["t"])
    nidx = np.asarray(inputs["neighbor_idx"]).astype(np.int32)
    x_full = np.concatenate([features, _sinusoidal(t)], -1)

    shared = _prep_shared(
        np.asarray(inputs["W1"], np.float32), np.asarray(inputs["b1"], np.float32),
        np.asarray(inputs["W2"], np.float32), np.asarray(inputs["b2"], np.float32),
        np.asarray(inputs["W3"], np.float32), np.asarray(inputs["b3"], np.float32),
        np.asarray(inputs["W4"], np.float32), np.asarray(inputs["b4"], np.float32),
    )
    in_maps = []
    for core in range(NCORES):
        m = _prep_core(core, x_full, nidx)
        m.update(shared)
        in_maps.append(m)

    nc = _get_nc()
    results, times = _run_pjrt(nc, in_maps, reps=reps)
    out = np.empty((N, 1), np.float32)
    for core in range(NCORES):
        out[core * PER:(core + 1) * PER, 0] = results[core]["out"][:PER]
    return out, times


def kernel(**inputs) -> np.ndarray:
    out, _ = _run(inputs, reps=0)
    return out


def _prep_in_maps(inputs):
    features = np.asarray(inputs["features"], np.float32)
    t = np.asarray(inputs["t"])
    nidx = np.asarray(inputs["neighbor_idx"]).astype(np.int32)
    x_full = np.concatenate([features, _sinusoidal(t)], -1)
    shared = _prep_shared(
        np.asarray(inputs["W1"], np.float32), np.asarray(inputs["b1"], np.float32),
        np.asarray(inputs["W2"], np.float32), np.asarray(inputs["b2"], np.float32),
        np.asarray(inputs["W3"], np.float32), np.asarray(inputs["b3"], np.float32),
        np.asarray(inputs["W4"], np.float32), np.asarray(inputs["b4"], np.float32),
    )
    in_maps = []
    for core in range(NCORES):
        m = _prep_core(core, x_full, nidx)
        m.update(shared)
        in_maps.append(m)
    return in_maps


def bench(inputs, loop_reps=(1, 26), wall_reps=8):
    """Estimate on-device kernel time by diffing wall times of programs that
    loop the whole body R1 vs R2 times on-device (cancels the ~105ms axon
    RPC floor)."""
    in_maps = _prep_in_maps(inputs)
    walls = {}
    outs = {}
    for R in loop_reps:
        nc = _build_program(bench_reps=R)
        results, times = _run_pjrt(nc, in_maps, reps=wall_reps)
        walls[R] = min(times)
        out = np.empty((N, 1), np.float32)
        for core in range(NCORES):
            out[core * PER:(core + 1) * PER, 0] = results[core]["out"][:PER]
        outs[R] = out
    R1, R2 = loop_reps
    per_iter = (walls[R2] - walls[R1]) / (R2 - R1)
    return per_iter, walls, outs
